# revision 6
# baseline (speedup 1.0000x reference)
"""Trainium2 Bass kernel for nn_MultiHeadAttention_3839700762945.

Full-shape contract: kernel(**inputs) takes the unsharded numpy inputs and
returns the full [4, 2048, 1024] output.

Sharding (8 cores): core c handles (batch b = c//2, head-half = c%2).
Each core computes q/k/v projections for its 8 heads (512 of the 1024 dim
columns) over the full sequence, runs attention for those heads, and emits a
partial output projection  OT_half.T @ Wo[half]  of shape [2048, 1024].
Host combines: out[b] = partial[2b] + partial[2b+1] + bo.  No collectives.

On-chip dataflow (per core, all matmuls in float32r = full-rate TF32-like):
  - Q/K/V are transposed on the PE (128x128 identity-transpose tiles) into
    [dim, seq] layout, rounded to f32r on eviction.
  - qT/kT are produced transposed ([d, s]) via lhsT=W chunks; v is produced
    natural ([s, d], bf16) with a ones-column appended for softmax row sums.
  - scoresT[sk, sq] = kT_h^T qT_h per head; exp via ScalarE (scale=1/8 folded
    in, no max-subtraction: scores ~ N(0,1), fp32 exp is safe), bf16 P tiles.
  - AV: psum[0:65] = [v_h | 1]^T @ P accumulated over sk; row 64 = softmax
    denominator.  Normalization via reciprocal + PE outer-product broadcast.
  - output projection from the transposed attention output (natural layout
    for lhsT) with Wo natural as moving operand.
"""

import sys

for _p in ("/opt/trn_rl_repo", "/opt/pypackages"):
    if _p not in sys.path:
        sys.path.insert(0, _p)

import numpy as np

import concourse.bass as bass
import concourse.mybir as mybir
import concourse.tile as tile
import concourse.bacc as bacc
from concourse import masks
from concourse.bass_utils import run_bass_kernel_spmd

F32 = mybir.dt.float32
F32R = mybir.dt.float32r
BF16 = mybir.dt.bfloat16
AF = mybir.ActivationFunctionType

B, S, DIM = 4, 2048, 1024
DH = 512          # dim columns per core (8 heads x 64)
NH = 8            # heads per core
HD = 64
P = 128
NKC = DIM // P    # 8 contraction chunks for projections
NMC = DH // P     # 4 output-dim chunks
NSK = S // P      # 16 sk chunks
BW = 256          # transpose/projection block width (seq cols per block)
NBLK = S // BW    # 8 blocks
SQT = 512         # attention query tile
NSQT = S // SQT   # 4
EG = 2            # exp group: sk chunks per ScalarE activation op
INV_SQRT_HD = 0.125


def _emit_input_phase(nc, pools, Xdram, Wdram, Bdram, kind, kT=None, vsb=None):
    """Transpose one input to [dim, seq] blocks and project it.

    kind: 'kq' -> write transposed projection into kT ([128, 4, 2048] f32r),
          'v'  -> write natural projection into vsb ([128, 16, 8, 66] bf16).
    """
    (pc, p2, p3, p4, ps_pp, ps_sc, ps_av) = pools

    # load + round weights (two halves through an 8KB staging tile)
    wsb = pc.tile([P, NKC, DH], F32R, tag="wproj")
    wview = Wdram.ap().rearrange("(kc p) d -> p kc d", p=P)
    for hw in range(2):
        wst = pc.tile([P, NKC // 2, DH], F32, tag="wstage")
        nc.sync.dma_start(wst[:], wview[:, hw * 4:(hw + 1) * 4, :])
        nc.vector.tensor_copy(wsb[:, hw * 4:(hw + 1) * 4, :], wst[:])

    # load + round bias row [1, 512]
    brow = pc.tile([1, DH], F32R, tag=f"brow_{kind}_{'v' if vsb is not None else 'kq'}")
    bst = pc.tile([1, DH], F32, tag="bstage")
    nc.sync.dma_start(bst[:], Bdram.ap())
    nc.vector.tensor_copy(brow[:], bst[:])

    ident = pools_consts["ident"]
    ones = pools_consts["ones"]
    Xap = Xdram.ap()

    for blk in range(NBLK):
        xts = p2.tile([P, NKC, BW], F32R, tag="xt")
        for j in range(2):
            xn = p3.tile([P, DIM], F32, tag="xnat")
            r0 = (blk * 2 + j) * P
            nc.sync.dma_start(xn[:], Xap[r0:r0 + P, :])
            for kq in range(2):
                pst = ps_pp.tile([P, 4, P], F32, tag="pp")
                for ki in range(4):
                    k = kq * 4 + ki
                    nc.tensor.transpose(
                        pst[:, ki, :], xn[:, k * P:(k + 1) * P], ident[:]
                    )
                nc.vector.tensor_copy(
                    xts[:, kq * 4:(kq + 1) * 4, j * P:(j + 1) * P], pst[:]
                )

        if kind == "kq":
            # out_T[d, sk] block: lhsT = W chunk (natural), rhs = X^T block
            for m in range(NMC):
                psp = ps_pp.tile([P, BW], F32, tag="pp")
                for k in range(NKC):
                    nc.tensor.matmul(
                        psp[:],
                        wsb[:, k, m * P:(m + 1) * P],
                        xts[:, k, :],
                        start=(k == 0),
                        stop=False,
                    )
                nc.tensor.matmul(
                    psp[:],
                    brow[0:1, m * P:(m + 1) * P],
                    ones[0:1, 0:BW],
                    start=False,
                    stop=True,
                )
                nc.vector.tensor_copy(
                    kT[:, m, blk * BW:(blk + 1) * BW], psp[:]
                )
        else:
            # v natural [sk, d]: lhsT = X^T chunk, rhs = W (moving, N=512)
            for j in range(2):
                c = blk * 2 + j
                psv = ps_pp.tile([P, DH], F32, tag="pp")
                for k in range(NKC):
                    nc.tensor.matmul(
                        psv[:],
                        xts[:, k, j * P:(j + 1) * P],
                        wsb[:, k, :],
                        start=(k == 0),
                        stop=False,
                    )
                nc.tensor.matmul(
                    psv[:],
                    ones[0:1, 0:P],
                    brow[0:1, :],
                    start=False,
                    stop=True,
                )
                nc.vector.tensor_copy(
                    vsb[:, c, :, 0:HD],
                    psv[:].rearrange("p (h d) -> p h d", h=NH),
                )


pools_consts = {}


def build_nc(reps: int = 1, mode: str = "full"):
    """Build the per-core Bass program (SPMD: all cores run this)."""
    nc = bacc.Bacc("TRN2", target_bir_lowering=False, debug=False, num_devices=8)

    XQ = nc.dram_tensor("XQ", (S, DIM), F32, kind="ExternalInput")
    XK = nc.dram_tensor("XK", (S, DIM), F32, kind="ExternalInput")
    XV = nc.dram_tensor("XV", (S, DIM), F32, kind="ExternalInput")
    WQ = nc.dram_tensor("WQ", (DIM, DH), F32, kind="ExternalInput")
    WK = nc.dram_tensor("WK", (DIM, DH), F32, kind="ExternalInput")
    WV = nc.dram_tensor("WV", (DIM, DH), F32, kind="ExternalInput")
    WO = nc.dram_tensor("WO", (DH, DIM), F32, kind="ExternalInput")
    BQ = nc.dram_tensor("BQ", (1, DH), F32, kind="ExternalInput")
    BK = nc.dram_tensor("BK", (1, DH), F32, kind="ExternalInput")
    BV = nc.dram_tensor("BV", (1, DH), F32, kind="ExternalInput")
    OUT = nc.dram_tensor("OUT", (S, DIM), F32, kind="ExternalOutput")

    with tile.TileContext(nc) as tc:
        with (
            tc.tile_pool(name="persist", bufs=1) as pc,
            tc.tile_pool(name="dbuf", bufs=2) as p2,
            tc.tile_pool(name="tri", bufs=3) as p3,
            tc.tile_pool(name="quad", bufs=4) as p4,
            tc.tile_pool(name="ps_pp", bufs=2, space="PSUM") as ps_pp,
            tc.tile_pool(name="ps_sc", bufs=2, space="PSUM") as ps_sc,
            tc.tile_pool(name="ps_av", bufs=2, space="PSUM") as ps_av,
        ):
            pools = (pc, p2, p3, p4, ps_pp, ps_sc, ps_av)

            # constants
            ident = pc.tile([P, P], F32, tag="ident")
            masks.make_identity(nc, ident[:])
            ones_f32 = pc.tile([1, BW], F32, tag="ones_st")
            nc.vector.memset(ones_f32[:], 1.0)
            ones = pc.tile([1, BW], F32R, tag="ones")
            nc.vector.tensor_copy(ones[:], ones_f32[:])
            pools_consts["ident"] = ident
            pools_consts["ones"] = ones

            for _rep in range(reps):
                # persistent per-rep tensors
                kT = pc.tile([P, NMC, S], F32R, tag="kT")
                qT = pc.tile([P, NMC, S], F32R, tag="qT")
                vsb = pc.tile([P, NSK, NH, HD + 2], BF16, tag="vsb")
                nc.vector.memset(vsb[:, :, :, HD:HD + 1], 1.0)

                wo_sb = pc.tile([P, NMC, DIM], F32R, tag="wo")
                woview = WO.ap().rearrange("(kc p) d -> p kc d", p=P)
                for hw in range(2):
                    wst = pc.tile([P, 2, DIM], F32, tag="wstage")
                    nc.sync.dma_start(wst[:], woview[:, hw * 2:(hw + 1) * 2, :])
                    nc.vector.tensor_copy(wo_sb[:, hw * 2:(hw + 1) * 2, :], wst[:])

                _emit_input_phase(nc, pools, XK, WK, BK, "kq", kT=kT)
                _emit_input_phase(nc, pools, XV, WV, BV, "v", vsb=vsb)

                # Q phase interleaved with attention per sq tile
                for sqt in range(NSQT):
                    _emit_q_blocks(nc, pools, XQ, WQ, BQ, qT, sqt)
                    if mode != "phase_a":
                        _emit_attention(nc, pools, kT, qT, vsb, wo_sb, OUT, sqt,
                                        mode)
                if mode == "phase_a":
                    # consume kT/qT/vsb so DCE keeps phase A
                    for m in range(NMC):
                        nc.sync.dma_start(
                            OUT.ap()[m * P:(m + 1) * P, 0:S // 2],
                            kT[:, m, 0:S // 2].bitcast(F32))
                        nc.sync.dma_start(
                            OUT.ap()[(4 + m) * P:(5 + m) * P, 0:S // 2],
                            qT[:, m, 0:S // 2].bitcast(F32))
                    vtmp = p2.tile([P, 512], F32, tag="vtmp")
                    nc.vector.tensor_copy(
                        vtmp[:],
                        vsb[:].rearrange("p a b c -> p (a b c)").bitcast(F32)[:, 0:512])
                    nc.sync.dma_start(OUT.ap()[1024:1024 + P, 0:512], vtmp[:])

    nc.compile()
    return nc


def _emit_q_blocks(nc, pools, XQ, WQ, BQ, qT, sqt):
    """Emit transpose+projection for the two 256-col Q blocks feeding sq tile
    `sqt` (cols sqt*512 .. sqt*512+512)."""
    (pc, p2, p3, p4, ps_pp, ps_sc, ps_av) = pools
    ident = pools_consts["ident"]
    ones = pools_consts["ones"]

    if sqt == 0:
        # weights + bias once
        wsb = pc.tile([P, NKC, DH], F32R, tag="wproj")
        wview = WQ.ap().rearrange("(kc p) d -> p kc d", p=P)
        for hw in range(2):
            wst = pc.tile([P, NKC // 2, DH], F32, tag="wstage")
            nc.sync.dma_start(wst[:], wview[:, hw * 4:(hw + 1) * 4, :])
            nc.vector.tensor_copy(wsb[:, hw * 4:(hw + 1) * 4, :], wst[:])
        brow = pc.tile([1, DH], F32R, tag="brow_q")
        bst = pc.tile([1, DH], F32, tag="bstage")
        nc.sync.dma_start(bst[:], BQ.ap())
        nc.vector.tensor_copy(brow[:], bst[:])
        pools_consts["wq_sb"] = wsb
        pools_consts["bq_row"] = brow
    wsb = pools_consts["wq_sb"]
    brow = pools_consts["bq_row"]
    Xap = XQ.ap()

    for blk in (2 * sqt, 2 * sqt + 1):
        xts = p2.tile([P, NKC, BW], F32R, tag="xt")
        for j in range(2):
            xn = p3.tile([P, DIM], F32, tag="xnat")
            r0 = (blk * 2 + j) * P
            nc.sync.dma_start(xn[:], Xap[r0:r0 + P, :])
            for kq in range(2):
                pst = ps_pp.tile([P, 4, P], F32, tag="pp")
                for ki in range(4):
                    k = kq * 4 + ki
                    nc.tensor.transpose(
                        pst[:, ki, :], xn[:, k * P:(k + 1) * P], ident[:]
                    )
                nc.vector.tensor_copy(
                    xts[:, kq * 4:(kq + 1) * 4, j * P:(j + 1) * P], pst[:]
                )
        for m in range(NMC):
            psp = ps_pp.tile([P, BW], F32, tag="pp")
            for k in range(NKC):
                nc.tensor.matmul(
                    psp[:],
                    wsb[:, k, m * P:(m + 1) * P],
                    xts[:, k, :],
                    start=(k == 0),
                    stop=False,
                )
            nc.tensor.matmul(
                psp[:],
                brow[0:1, m * P:(m + 1) * P],
                ones[0:1, 0:BW],
                start=False,
                stop=True,
            )
            nc.vector.tensor_copy(qT[:, m, blk * BW:(blk + 1) * BW], psp[:])


def _emit_attention(nc, pools, kT, qT, vsb, wo_sb, OUT, sqt, mode="full"):
    (pc, p2, p3, p4, ps_pp, ps_sc, ps_av) = pools
    ones = pools_consts["ones"]
    sq0 = sqt * SQT

    ot = p2.tile([P, NMC, SQT], F32R, tag="ot")
    NG = NSK // EG
    total = NH * NG
    psavs = {}
    ptts = {}

    def emit_normalize(h):
        base = (h % 2) * HD
        mch = h // 2
        psav = psavs.pop(h)
        orw = p4.tile([HD + 1, SQT], F32, tag="oraw")
        nc.vector.tensor_copy(orw[:], psav[0:HD + 1, :])
        rc = p2.tile([1, SQT], F32R, tag="recip")
        with nc.allow_low_precision("softmax denominator broadcast"):
            nc.vector.reciprocal(rc[:], orw[HD:HD + 1, :])
        psb = ps_pp.tile([P, SQT], F32, tag="pp")
        nc.tensor.matmul(
            psb[0:HD, :], ones[0:1, 0:HD], rc[:], start=True, stop=True
        )
        bcs = p2.tile([HD, SQT], F32, tag="bc")
        nc.vector.tensor_copy(bcs[:], psb[0:HD, :])
        nc.vector.tensor_mul(ot[base:base + HD, mch, :], orw[0:HD, :], bcs[:])

    # software pipeline: scores/exp for group idx, AV for group idx-1 —
    # keeps ScalarE (exp) saturated; PE never sits between exp and AV.
    for idx in range(total + 1):
        if idx < total:
            h, g = divmod(idx, NG)
            base = (h % 2) * HD
            mch = h // 2
            pss = ps_sc.tile([P, EG, SQT], F32, tag="sc")
            for ci in range(EG):
                c = g * EG + ci
                nc.tensor.matmul(
                    pss[:, ci, :],
                    kT[base:base + HD, mch, c * P:(c + 1) * P],
                    qT[base:base + HD, mch, sq0:sq0 + SQT],
                    start=True,
                    stop=True,
                )
            ptt = p4.tile([P, EG, SQT], BF16, tag="pt")
            if mode == "noexp":
                nc.vector.tensor_copy(ptt[:], pss[:])
            else:
                nc.scalar.activation(ptt[:], pss[:], AF.Exp, scale=INV_SQRT_HD)
            ptts[idx] = ptt
        if idx >= 1:
            h2, g2 = divmod(idx - 1, NG)
            if g2 == 0:
                psavs[h2] = ps_av.tile([P, SQT], F32, tag="av", name="psav")
            ptt2 = ptts.pop(idx - 1)
            for ci in range(EG):
                c = g2 * EG + ci
                nc.tensor.matmul(
                    psavs[h2][0:HD + 1, :],
                    vsb[:, c, h2, 0:HD + 1],
                    ptt2[:, ci, :],
                    start=(c == 0),
                    stop=(c == NSK - 1),
                )
            if g2 == NG - 1:
                emit_normalize(h2)

    # output projection for this sq tile: out[sq, :] = ot^T @ Wo (partial)
    for m in range(NMC):
        ostg = p2.tile([P, 2, DH], F32, tag="ostg")
        for n2 in range(2):
            pso = ps_pp.tile([P, DH], F32, tag="pp")
            for k in range(NMC):
                nc.tensor.matmul(
                    pso[:],
                    ot[:, k, m * P:(m + 1) * P],
                    wo_sb[:, k, n2 * DH:(n2 + 1) * DH],
                    start=(k == 0),
                    stop=(k == NMC - 1),
                )
            nc.vector.tensor_copy(ostg[:, n2, :], pso[:])
        r0 = sq0 + m * P
        nc.sync.dma_start(
            OUT.ap()[r0:r0 + P, :].rearrange("p (n d) -> p n d", n=2), ostg[:]
        )


_cached = {}


def _get_nc(reps: int = 1, mode: str = "full"):
    key = (reps, mode)
    if key not in _cached:
        _cached[key] = build_nc(reps, mode)
    return _cached[key]


def make_in_maps(Q, K, V, Wq, bq, Wk, bk, Wv, bv, Wo, bo):
    asf = lambda x: np.ascontiguousarray(np.asarray(x, dtype=np.float32))
    in_maps = []
    for c in range(8):
        b, half = divmod(c, 2)
        sl = slice(half * DH, (half + 1) * DH)
        in_maps.append({
            "XQ": asf(Q[b]),
            "XK": asf(K[b]),
            "XV": asf(V[b]),
            "WQ": asf(Wq[:, sl]),
            "WK": asf(Wk[:, sl]),
            "WV": asf(Wv[:, sl]),
            "WO": asf(Wo[sl, :]),
            "BQ": asf(bq[sl]).reshape(1, DH),
            "BK": asf(bk[sl]).reshape(1, DH),
            "BV": asf(bv[sl]).reshape(1, DH),
        })
    return in_maps


def combine(results, bo):
    bo = np.asarray(bo, dtype=np.float32)
    return np.stack([
        results[2 * b]["OUT"] + results[2 * b + 1]["OUT"] + bo
        for b in range(B)
    ])


def kernel(Q, K, V, Wq, bq, Wk, bk, Wv, bv, Wo, bo):
    nc = _get_nc(1)
    in_maps = make_in_maps(Q, K, V, Wq, bq, Wk, bk, Wv, bv, Wo, bo)
    res = run_bass_kernel_spmd(nc, in_maps, core_ids=list(range(8)))
    return combine(res.results, bo)
